# revision 42
# baseline (speedup 1.0000x reference)
"""Trainium2 Bass kernel for nn_AttentionBlock (B=4, S=2048, D=1024, DQK=256).

Sharding: 8 cores = 4 batches x 2 query-halves. Each core computes K/V for its
full batch (duplicated across the pair) and attention for its own 1024 queries.
SPMD trick: each core's x is passed feature-major with its own query half
rotated to the front, so one program serves all cores.

FP8 pipeline (e4m3 DoubleRow = 0.5 cycles/row, 4x the fp32r rate):
  - Host splits x and the x32-scaled weights into e4m3 hi/lo pairs. Q/K/V
    projections run as 3-term compensated fp8 (hi*hi + hi*lo + lo*hi), 0.75x
    the fp32r cost with ~2e-3 relative error.
  - Scores stay fp32r on f32 Q^T/K^T (exp amplifies score errors).
  - P_hat = e4m3(64 * P / l): the per-row rescale keeps every row's top
    attention weight inside e4m3's normal range regardless of its row max
    (row maxes span ~[13, 29] here, 23 octaves > e4m3's 12). The scale is
    exactly canceled by dividing by l_hat = sum_k P_hat (computed from the
    *rounded* P_hat so rounding errors cancel too).
  - attended = P_hat @ (V_hi + V_lo) with V split e4m3 hi/lo on device:
    0.5x the fp32r cost.
  - bv never exists on device: host sends resid = x_q + bv, and the 1/32
    weight scale folds into the constant-32 ones column of the l_hat matmul.
All inputs arrive in partition-major consolidated layouts (one big DMA per
tensor: the DMA cost is dominated by a per-transfer floor, not bytes). The
two q-blocks are software-pipelined: q-block 1's scores/exp run on PE/ACT
while q-block 0's attended matmuls execute.
Measured end-to-end numerics on the real inputs: ~4.7e-3 L2 rel err.
"""
import os
import tempfile

# The neuron compile cache keys are not content-unique across different bass
# kernels (the BIR rides in backend_config, outside the module hash), so a
# shared cache can silently serve a stale NEFF. Use a private empty cache dir.
os.environ["NEURON_COMPILE_CACHE_URL"] = tempfile.mkdtemp(prefix="neff_cache_")

import numpy as np

B, S, D = 4, 2048, 1024
DQK = D // 4
H = S // 2          # queries per core
N_CORES = 8
EXP_SHIFT = 40.0    # max unscaled score over these inputs is ~34.6
WSC = 32.0          # weight scale so e4m3 lo-splits stay in normal range

_RUNNER = None


def _build_kernel(reps=1, salt=3):
    from concourse import bacc
    import concourse.tile as tile
    import concourse.mybir as mybir

    F = mybir.dt.float32
    R = mybir.dt.float32r
    BF = mybir.dt.bfloat16
    E4 = mybir.dt.float8e4
    DR = mybir.MatmulPerfMode.DoubleRow
    Alu = mybir.AluOpType

    nc = bacc.Bacc(None, debug=False)

    # consolidated partition-major layouts (a few big DMAs; see host_inputs)
    xh = nc.declare_dram_parameter("xh", [128, 8, S], E4, isOutput=False)
    xl = nc.declare_dram_parameter("xl", [128, 8, S], E4, isOutput=False)
    wqh = nc.declare_dram_parameter("wqh", [128, 8, DQK], E4, isOutput=False)
    wql = nc.declare_dram_parameter("wql", [128, 8, DQK], E4, isOutput=False)
    wkh = nc.declare_dram_parameter("wkh", [128, 8, DQK], E4, isOutput=False)
    wkl = nc.declare_dram_parameter("wkl", [128, 8, DQK], E4, isOutput=False)
    wvh = nc.declare_dram_parameter("wvh", [128, 8, D], E4, isOutput=False)
    wvl = nc.declare_dram_parameter("wvl", [128, 8, D], E4, isOutput=False)
    bqk = nc.declare_dram_parameter("bqk", [DQK, 2], F, isOutput=False)
    resid = nc.declare_dram_parameter("resid", [128, 8, D], F, isOutput=False)
    onescol = nc.declare_dram_parameter("onescol", [128, 1], BF, isOutput=False)
    ones1p = nc.declare_dram_parameter("ones1p", [1, 128], BF, isOutput=False)
    ones32 = nc.declare_dram_parameter("ones32", [128, 4], E4, isOutput=False)
    # salt: dummy input whose shape makes each build's HLO structurally unique,
    # defeating executable dedup layers that ignore backend_config
    salt_p = nc.declare_dram_parameter("salt", [1, salt], F, isOutput=False)
    out = nc.declare_dram_parameter("out", [H, D], F, isOutput=True)

    NJ = D // 256     # 4 contraction pair-tiles (2x128 rows each)
    NE = DQK // 128   # 2 e-tiles
    NK = S // 128     # 16 k-tiles
    NM = NK // 2      # 8 k pair-tiles
    QB = 512          # q-block
    NQB = H // QB     # 2 q-blocks per core
    NQT = QB // 128   # 4 q-tiles per block

    with tile.TileContext(nc) as tc:
        with (
            tc.tile_pool(name="consts", bufs=1) as cp,
            tc.tile_pool(name="qt_sb", bufs=NE) as qtp,
            tc.tile_pool(name="kt_sb", bufs=NE) as ktp,
            tc.tile_pool(name="vh_sb", bufs=NM) as vhp,
            tc.tile_pool(name="vl_sb", bufs=NM) as vlp,
        ):
            nbias = cp.tile([128, 1], F, tag="nbias")
            nc.gpsimd.memset(nbias[:], -EXP_SHIFT)
            # dummy exp right away so the ACT table load happens at t=0, not
            # in the middle of the first scores->exp stream
            scratch1 = cp.tile([128, 1], F, tag="scratch1")
            nc.scalar.activation(
                scratch1[:], nbias[:], mybir.ActivationFunctionType.Exp
            )
            ones_col = cp.tile([128, 1], BF, tag="ones_col")
            nc.sync.dma_start(ones_col[:], onescol[:])
            ones_row = cp.tile([1, 128], BF, tag="ones_row")
            nc.sync.dma_start(ones_row[:], ones1p[:])
            ones32_sb = cp.tile([128, 2, 2], E4, tag="ones32")
            nc.sync.dma_start(ones32_sb[:, :, :], ones32[:, :])
            bqk_sb = cp.tile([128, 2, 2], F, tag="bqk")
            for e in range(NE):
                nc.sync.dma_start(bqk_sb[:, e : e + 1, :], bqk[e * 128 : (e + 1) * 128, :])
            salt_sb = cp.tile([1, salt], F, tag="salt")
            nc.sync.dma_start(salt_sb[:], salt_p[:])

            QT = [qtp.tile([128, H], R, tag="qt", name=f"QT{e}") for e in range(NE)]
            KT = [ktp.tile([128, S], R, tag="kt", name=f"KT{e}") for e in range(NE)]
            # V pair tiles: dim1 = k-tile parity for DoubleRow pairing
            Vh = [vhp.tile([128, 2, D], E4, tag="vh", name=f"Vh{m}") for m in range(NM)]
            Vl = [vlp.tile([128, 2, D], E4, tag="vl", name=f"Vl{m}") for m in range(NM)]

            def jsl(big, j, c0, c1):
                return big[:, 2 * j : 2 * j + 2, c0:c1]

            def load_inputs(xsp, wsp):
                """All inputs as a few big partition-major DMAs, ordered by
                first use: wq, first x chunk, wq_lo, ... (QKT runs first),
                then the rest of x, then Wv."""
                xhs = xsp.tile([128, 8, S], E4, tag="xh", name="xhs")
                xls = xsp.tile([128, 8, S], E4, tag="xl", name="xls")
                wvhs = wsp.tile([128, 8, D], E4, tag="wvh", name="wvhs")
                wvls = wsp.tile([128, 8, D], E4, tag="wvl", name="wvls")

                def chunk(dst, src, c0, c1):
                    nc.sync.dma_start(dst[:, :, c0:c1], src[:, :, c0:c1])

                wqhs = wsp.tile([128, 8, DQK], E4, tag="wqh", name="wqhs")
                nc.sync.dma_start(wqhs[:, :, :], wqh[:, :, :])
                chunk(xhs, xh, 0, 512)
                wqls = wsp.tile([128, 8, DQK], E4, tag="wql", name="wqls")
                nc.sync.dma_start(wqls[:, :, :], wql[:, :, :])
                chunk(xls, xl, 0, 512)
                wkhs = wsp.tile([128, 8, DQK], E4, tag="wkh", name="wkhs")
                nc.sync.dma_start(wkhs[:, :, :], wkh[:, :, :])
                wkls = wsp.tile([128, 8, DQK], E4, tag="wkl", name="wkls")
                nc.sync.dma_start(wkls[:, :, :], wkl[:, :, :])
                for sb in range(1, 4):
                    chunk(xhs, xh, sb * 512, (sb + 1) * 512)
                    chunk(xls, xl, sb * 512, (sb + 1) * 512)
                for vb in range(2):
                    chunk(wvhs, wvh, vb * 512, (vb + 1) * 512)
                    chunk(wvls, wvl, vb * 512, (vb + 1) * 512)
                return xhs, xls, wqhs, wqls, wkhs, wkls, wvhs, wvls

            def proj_qkt(x2, w4, pqkp):
                """QT = (x[:H] @ Wq')^T + bq', KT = (x @ Wk')^T + bk'.
                blk-major so the x-chunk DMA stream stays ahead of the PE."""
                xhs, xls = x2
                wqhs, wqls, wkhs, wkls = w4
                for dst, nblk, wh_, wl_, bsel in (
                    (QT, NQB, wqhs, wqls, 0),
                    (KT, S // 512, wkhs, wkls, 1),
                ):
                    for blk in range(nblk):
                        for e in range(NE):
                            ps = pqkp.tile([128, 512], F, tag="pqk")
                            c = 0
                            for xs, ws in ((xhs, wh_), (xhs, wl_), (xls, wh_)):
                                for j in range(NJ):
                                    nc.tensor.matmul(
                                        ps[:],
                                        jsl(ws, j, e * 128, (e + 1) * 128),
                                        jsl(xs, j, blk * 512, (blk + 1) * 512),
                                        start=(c == 0),
                                        stop=(c == 3 * NJ - 1),
                                        perf_mode=DR,
                                    )
                                    c += 1
                            nc.vector.tensor_scalar_add(
                                dst[e][:, blk * 512 : (blk + 1) * 512],
                                ps[:],
                                bqk_sb[:, e : e + 1, bsel : bsel + 1].opt(),
                            )

            def compute_v(x2, wv2, pvp):
                """V = x @ Wv' (no bias: bv lives in resid), split e4m3 hi/lo.
                The V_lo subtract runs on the otherwise-idle Pool engine."""
                xhs, xls = x2
                wvhs, wvls = wv2
                for vb in range(2):
                    for kt in range(NK):
                        m, par = kt // 2, kt % 2
                        ps = pvp.tile([128, 512], F, tag="pv")
                        c = 0
                        for xs, ws in ((xhs, wvhs), (xhs, wvls), (xls, wvhs)):
                            for j in range(NJ):
                                nc.tensor.matmul(
                                    ps[:],
                                    jsl(xs, j, kt * 128, (kt + 1) * 128),
                                    jsl(ws, j, vb * 512, (vb + 1) * 512),
                                    start=(c == 0),
                                    stop=(c == 3 * NJ - 1),
                                    perf_mode=DR,
                                )
                                c += 1
                        vsl = (slice(None), slice(par, par + 1),
                               slice(vb * 512, (vb + 1) * 512))
                        nc.scalar.activation(
                            Vh[m][vsl], ps[:], mybir.ActivationFunctionType.Copy
                        )
                        nc.vector.scalar_tensor_tensor(
                            out=Vl[m][vsl],
                            in0=ps[:],
                            scalar=1.0,
                            in1=Vh[m][vsl],
                            op0=Alu.mult,
                            op1=Alu.subtract,
                        )

            def stage_scores(qb, pbfp, lrp, pst, prow):
                """scores -> exp -> l_row -> linv for one q-block."""
                pbf = []
                l_bank = prow.tile([128, 512], F, tag="rowbank", name=f"lbank{qb}")
                l_ps = l_bank[0:1, :]

                def l_row(kt):
                    nc.tensor.matmul(
                        l_ps,
                        ones_col[:],
                        pbf[kt][:],
                        start=(kt == 0),
                        stop=(kt == NK - 1),
                    )

                for kt in range(NK):
                    ps = pst.tile([128, QB], F, tag="st")
                    for e in range(NE):
                        nc.tensor.matmul(
                            ps[:],
                            KT[e][:, kt * 128 : (kt + 1) * 128],
                            QT[e][:, qb * QB : (qb + 1) * QB],
                            start=(e == 0),
                            stop=(e == NE - 1),
                        )
                    pt = pbfp.tile([128, QB], BF, tag="pbf", name=f"pbf{qb}_{kt}")
                    nc.scalar.activation(
                        pt[:],
                        ps[:],
                        mybir.ActivationFunctionType.Exp,
                        bias=nbias[:],
                        scale=1.0 / (WSC * WSC),
                    )
                    pbf.append(pt)
                    # skew the l_row accumulation one tile behind the scores
                    # so PE never waits on ACT mid-stream
                    if kt >= 1:
                        l_row(kt - 1)
                l_row(NK - 1)
                linv_row = lrp.tile([1, 512], F, tag="linv")
                nc.vector.reciprocal_approx_fast(linv_row[:], l_ps)
                linv_bf = lrp.tile([1, 512], BF, tag="linvbf")
                nc.vector.tensor_copy(out=linv_bf[:], in_=linv_row[:])
                return pbf, linv_bf

            def stage_phat(qb, pbf, linv_row, phpp, prow, lrp):
                """rank-1 broadcast of linv via the PE, then
                P_hat = e4m3(64 * P * linv) as DoubleRow pair tiles on the
                otherwise-idle Pool engine (via an SBUF copy of the
                broadcast: Pool cannot read PSUM)."""
                bc_ps = prow.tile([128, 512], F, tag="rowbank", name=f"bcbank{qb}")
                nc.tensor.matmul(bc_ps[:], ones_row[:], linv_row[:])
                php = [phpp.tile([128, 2, QB], E4, tag="php", name=f"php{qb}_{m}")
                       for m in range(NM)]
                for kt in range(NK):
                    nc.vector.scalar_tensor_tensor(
                        out=php[kt // 2][:, kt % 2 : kt % 2 + 1, :],
                        in0=pbf[kt][:],
                        scalar=64.0,
                        in1=bc_ps[:],
                        op0=Alu.mult,
                        op1=Alu.mult,
                    )
                return php

            def stage_b(qb, php, op, lip, plh, patt, resid_sb):
                """l_hat + attended + output for one q-block. The final
                normalize+residual runs on the Pool engine."""
                for qt in range(NQT):
                    qtg = qb * NQT + qt
                    qsl = (slice(None), slice(None), slice(qt * 128, (qt + 1) * 128))
                    # l_hat = 32 * sum_k P_hat  (the 32 folds away 1/WSC);
                    # interleaved with the hi-term attended matmuls so each
                    # loaded P_hat stationary serves 3 matmuls
                    lh_ps = plh.tile([128, 2], F, tag="lh")
                    att = [
                        patt.tile([128, 512], F, tag="att", name=f"att{vb}")
                        for vb in range(2)
                    ]
                    for m in range(NM):
                        nc.tensor.matmul(
                            lh_ps[:],
                            php[m][qsl],
                            ones32_sb[:],
                            start=(m == 0),
                            stop=(m == NM - 1),
                            perf_mode=DR,
                        )
                        for Vp in (Vh, Vl):
                            for vb in range(2):
                                nc.tensor.matmul(
                                    att[vb][:],
                                    php[m][qsl],
                                    Vp[m][:, :, vb * 512 : (vb + 1) * 512],
                                    start=(m == 0 and Vp is Vh),
                                    stop=(m == NM - 1 and Vp is Vl),
                                    perf_mode=DR,
                                )
                    l2inv = lip.tile([128, 1], F, tag="l2inv")
                    nc.vector.reciprocal(l2inv[:], lh_ps[:, 0:1])
                    o_t = op.tile([128, D], F, tag="o")
                    for vb in range(2):
                        nc.vector.scalar_tensor_tensor(
                            out=o_t[:, vb * 512 : (vb + 1) * 512],
                            in0=att[vb][:],
                            scalar=l2inv[:],
                            in1=resid_sb[:, qtg : qtg + 1, vb * 512 : (vb + 1) * 512].opt(),
                            op0=Alu.mult,
                            op1=Alu.add,
                        )
                    nc.sync.dma_start(
                        out[qtg * 128 : (qtg + 1) * 128, :], o_t[:]
                    )

            for _rep in range(reps):
                if _rep > 0:
                    tc.strict_bb_all_engine_barrier()
                with (
                    tc.tile_pool(name="x_sb", bufs=1) as xsp,
                    tc.tile_pool(name="w_sb", bufs=1) as wsp,
                    tc.tile_pool(name="resid_sb", bufs=1) as rsp,
                    tc.tile_pool(name="pbf_sb", bufs=2 * NK) as pbfp,
                    tc.tile_pool(name="php_sb", bufs=2 * NM) as phpp,
                    tc.tile_pool(name="linv_sb", bufs=2) as lrp,
                    tc.tile_pool(name="o_sb", bufs=2) as op,
                    tc.tile_pool(name="l2_sb", bufs=2) as lip,
                ):
                    ins = load_inputs(xsp, wsp)
                    xhs, xls, wqhs, wqls, wkhs, wkls, wvhs, wvls = ins
                    resid_t = rsp.tile([128, 8, D], F, tag="resid", name="resids")
                    nc.sync.dma_start(resid_t[:, :, :], resid[:, :, :])
                    with tc.tile_pool(name="pqk", bufs=2, space="PSUM") as pqkp:
                        proj_qkt((xhs, xls), (wqhs, wqls, wkhs, wkls), pqkp)
                    with tc.tile_pool(name="prow", bufs=2, space="PSUM") as prow:
                        with tc.tile_pool(name="pst", bufs=3, space="PSUM") as pst:
                            pbf0, linv0 = stage_scores(0, pbfp, lrp, pst, prow)
                            pbf1, linv1 = stage_scores(1, pbfp, lrp, pst, prow)
                        # P_hat conversions (DVE) overlap the V matmuls below
                        php0 = stage_phat(0, pbf0, linv0, phpp, prow, lrp)
                        php1 = stage_phat(1, pbf1, linv1, phpp, prow, lrp)
                        with tc.tile_pool(name="pv", bufs=4, space="PSUM") as pvp:
                            compute_v((xhs, xls), (wvhs, wvls), pvp)
                    with (
                        tc.tile_pool(name="plh", bufs=1, space="PSUM") as plh,
                        tc.tile_pool(name="patt", bufs=4, space="PSUM") as patt,
                    ):
                        stage_b(0, php0, op, lip, plh, patt, resid_t)
                        stage_b(1, php1, op, lip, plh, patt, resid_t)

    nc.finalize()
    return nc


class _SpmdRunner:
    """Run a finalized Bass module on n_cores via PJRT (axon path)."""

    def __init__(self, nc, n_cores):
        import jax
        from jax.sharding import Mesh, PartitionSpec

        try:
            from jax.experimental.shard_map import shard_map
        except ImportError:
            from jax.shard_map import shard_map
        import concourse.mybir as mybir
        from concourse.bass2jax import (
            _bass_exec_p,
            install_neuronx_cc_hook,
            partition_id_tensor,
        )

        install_neuronx_cc_hook()
        self.jax = jax
        self.n_cores = n_cores
        partition_name = (
            nc.partition_id_tensor.name if nc.partition_id_tensor else None
        )
        in_names, out_names, out_avals, zero_outs = [], [], [], []
        for alloc in nc.m.functions[0].allocations:
            if not isinstance(alloc, mybir.MemoryLocationSet):
                continue
            name = alloc.memorylocations[0].name
            if alloc.kind == "ExternalInput":
                if name != partition_name:
                    in_names.append(name)
            elif alloc.kind == "ExternalOutput":
                out_names.append(name)
                shape = tuple(alloc.tensor_shape)
                dtype = mybir.dt.np(alloc.dtype)
                out_avals.append(jax.core.ShapedArray(shape, dtype))
                zero_outs.append(np.zeros(shape, dtype))
        self.in_names = in_names
        self.out_names = out_names
        self.out_avals = out_avals
        self.zero_outs = zero_outs
        n_params = len(in_names)
        n_outs = len(out_avals)
        all_in_names = list(in_names) + list(out_names)
        if partition_name is not None:
            all_in_names.append(partition_name)

        def _body(*args):
            operands = list(args)
            if partition_name is not None:
                operands.append(partition_id_tensor())
            outs = _bass_exec_p.bind(
                *operands,
                out_avals=tuple(out_avals),
                in_names=tuple(all_in_names),
                out_names=tuple(out_names),
                lowering_input_output_aliases=(),
                sim_require_finite=True,
                sim_require_nnan=True,
                nc=nc,
            )
            return tuple(outs)

        donate = tuple(range(n_params, n_params + n_outs))
        devices = jax.devices()[:n_cores]
        assert len(devices) == n_cores, (
            f"need {n_cores} devices, found {len(jax.devices())}"
        )
        mesh = Mesh(np.asarray(devices), ("core",))
        in_specs = (PartitionSpec("core"),) * (n_params + n_outs)
        out_specs = (PartitionSpec("core"),) * n_outs
        self.fn = jax.jit(
            shard_map(
                _body,
                mesh=mesh,
                in_specs=in_specs,
                out_specs=out_specs,
                check_rep=False,
            ),
            donate_argnums=donate,
            keep_unused=True,
        )

    def set_inputs(self, in_maps):
        n = len(self.in_names)
        per_core = [
            [np.ascontiguousarray(m[name]) for name in self.in_names]
            for m in in_maps
        ]
        concat_in = [
            np.concatenate([per_core[c][i] for c in range(self.n_cores)], axis=0)
            for i in range(n)
        ]
        self.dev_in = [self.jax.device_put(a) for a in concat_in]
        self.jax.block_until_ready(self.dev_in)

    def run(self, reuse_out=None):
        if reuse_out is None:
            outs = [
                np.zeros((self.n_cores * z.shape[0], *z.shape[1:]), z.dtype)
                for z in self.zero_outs
            ]
        else:
            outs = reuse_out
        outs = self.fn(*self.dev_in, *outs)
        self.jax.block_until_ready(outs)
        self._last = outs
        return outs

    def results(self):
        return [
            {
                name: np.asarray(self._last[i]).reshape(
                    self.n_cores, *self.out_avals[i].shape
                )[c]
                for i, name in enumerate(self.out_names)
            }
            for c in range(self.n_cores)
        ]


def _get_runner():
    global _RUNNER
    if _RUNNER is None:
        last = None
        for _attempt in range(3):
            try:
                nc = _build_kernel()
                break
            except Exception as e:  # rare Tile-scheduler deadlock flake
                last = e
        else:
            raise last
        _RUNNER = _SpmdRunner(nc, N_CORES)
    return _RUNNER


def _e4(a):
    import ml_dtypes

    return np.asarray(a, dtype=ml_dtypes.float8_e4m3)


def _split8(a):
    hi = _e4(a)
    lo = _e4(np.asarray(a, np.float32) - hi.astype(np.float32))
    return hi, lo


def _pmaj(a):
    """[128*k, n] -> [128, k, n] partition-major consolidation."""
    k = a.shape[0] // 128
    return np.ascontiguousarray(a.reshape(k, 128, a.shape[1]).transpose(1, 0, 2))


def host_inputs(x, Wq, bq, Wk, bk, Wv, bv, salt=3):
    """Build the per-core input maps (also used by test.py)."""
    import ml_dtypes

    x = np.ascontiguousarray(np.asarray(x, dtype=np.float32))
    Wq = np.asarray(Wq, np.float32) * WSC
    Wk = np.asarray(Wk, np.float32) * WSC
    Wv = np.asarray(Wv, np.float32) * WSC
    bqk = np.stack(
        [np.asarray(bq, np.float32) * WSC, np.asarray(bk, np.float32) * WSC], axis=1
    )  # [DQK, 2]
    bv = np.asarray(bv, np.float32)

    wq_hi, wq_lo = (_pmaj(w) for w in _split8(Wq))
    wk_hi, wk_lo = (_pmaj(w) for w in _split8(Wk))
    wv_hi, wv_lo = (_pmaj(w) for w in _split8(Wv))
    x_hi, x_lo = _split8(x)   # [B, S, D]

    onescol = np.ones((128, 1), ml_dtypes.bfloat16)
    ones1p = np.ones((1, 128), ml_dtypes.bfloat16)
    ones32 = np.full((128, 4), 32.0, ml_dtypes.float8_e4m3)
    saltz = np.zeros((1, salt), np.float32)

    in_maps = []
    for c in range(N_CORES):
        b, h = c // 2, c % 2
        rot = np.concatenate(
            [np.arange(h * H, (h + 1) * H), np.arange((1 - h) * H, (2 - h) * H)]
        )
        in_maps.append(
            {
                "xh": _pmaj(np.ascontiguousarray(x_hi[b][rot].T)),
                "xl": _pmaj(np.ascontiguousarray(x_lo[b][rot].T)),
                "wqh": wq_hi, "wql": wq_lo,
                "wkh": wk_hi, "wkl": wk_lo,
                "wvh": wv_hi, "wvl": wv_lo,
                "bqk": bqk,
                "resid": _pmaj(x[b, h * H : (h + 1) * H] + bv),
                "onescol": onescol, "ones1p": ones1p, "ones32": ones32,
                "salt": saltz,
            }
        )
    return in_maps


def kernel(x, Wq, bq, Wk, bk, Wv, bv):
    in_maps = host_inputs(x, Wq, bq, Wk, bk, Wv, bv)
    runner = _get_runner()
    runner.set_inputs(in_maps)
    runner.run()
    res = runner.results()
    outp = np.empty((B, S, D), np.float32)
    for c in range(N_CORES):
        b, h = c // 2, c % 2
        outp[b, h * H : (h + 1) * H] = res[c]["out"]
    return outp


# revision 49
# speedup vs baseline: 1.0197x; 1.0197x over previous
"""Trainium2 Bass kernel for nn_AttentionBlock (B=4, S=2048, D=1024, DQK=256).

Sharding: 8 cores = 4 batches x 2 query-halves. Each core computes K/V for its
full batch (duplicated across the pair) and attention for its own 1024 queries.
SPMD trick: each core's x is passed feature-major with its own query half
rotated to the front, so one program serves all cores.

FP8 pipeline (e4m3 DoubleRow = 0.5 cycles/row, 4x the fp32r rate):
  - Host splits x and the x32-scaled weights into e4m3 hi/lo pairs. Q/K/V
    projections run as 3-term compensated fp8 (hi*hi + hi*lo + lo*hi), 0.75x
    the fp32r cost with ~2e-3 relative error.
  - Scores stay fp32r on f32 Q^T/K^T (exp amplifies score errors).
  - P_hat = e4m3(64 * P / l): the per-row rescale keeps every row's top
    attention weight inside e4m3's normal range regardless of its row max
    (row maxes span ~[13, 29] here, 23 octaves > e4m3's 12). The scale is
    exactly canceled by dividing by l_hat = sum_k P_hat (computed from the
    *rounded* P_hat so rounding errors cancel too).
  - attended = P_hat @ (V_hi + V_lo) with V split e4m3 hi/lo on device:
    0.5x the fp32r cost.
  - bv never exists on device: host sends resid = x_q + bv, and the 1/32
    weight scale folds into the constant-32 ones column of the l_hat matmul.
All inputs arrive in partition-major consolidated layouts (one big DMA per
tensor: the DMA cost is dominated by a per-transfer floor, not bytes). The
two q-blocks are software-pipelined: q-block 1's scores/exp run on PE/ACT
while q-block 0's attended matmuls execute.
Measured end-to-end numerics on the real inputs: ~4.7e-3 L2 rel err.
"""
import os
import tempfile

# The neuron compile cache keys are not content-unique across different bass
# kernels (the BIR rides in backend_config, outside the module hash), so a
# shared cache can silently serve a stale NEFF. Use a private empty cache dir.
os.environ["NEURON_COMPILE_CACHE_URL"] = tempfile.mkdtemp(prefix="neff_cache_")

import numpy as np

B, S, D = 4, 2048, 1024
DQK = D // 4
H = S // 2          # queries per core
N_CORES = 8
EXP_SHIFT = 40.0    # max unscaled score over these inputs is ~34.6
WSC = 32.0          # weight scale so e4m3 lo-splits stay in normal range

_RUNNER = None


def _build_kernel(reps=1, salt=3):
    from concourse import bacc
    import concourse.tile as tile
    import concourse.mybir as mybir

    F = mybir.dt.float32
    R = mybir.dt.float32r
    BF = mybir.dt.bfloat16
    E4 = mybir.dt.float8e4
    DR = mybir.MatmulPerfMode.DoubleRow
    Alu = mybir.AluOpType

    nc = bacc.Bacc(None, debug=False)

    # consolidated partition-major layouts (a few big DMAs; see host_inputs)
    xh = nc.declare_dram_parameter("xh", [128, 8, S], E4, isOutput=False)
    xl = nc.declare_dram_parameter("xl", [128, 8, S], E4, isOutput=False)
    wqh = nc.declare_dram_parameter("wqh", [128, 8, DQK], E4, isOutput=False)
    wql = nc.declare_dram_parameter("wql", [128, 8, DQK], E4, isOutput=False)
    wkh = nc.declare_dram_parameter("wkh", [128, 8, DQK], E4, isOutput=False)
    wkl = nc.declare_dram_parameter("wkl", [128, 8, DQK], E4, isOutput=False)
    wvh = nc.declare_dram_parameter("wvh", [128, 8, D], E4, isOutput=False)
    wvl = nc.declare_dram_parameter("wvl", [128, 8, D], E4, isOutput=False)
    bqk = nc.declare_dram_parameter("bqk", [DQK, 2], F, isOutput=False)
    resid = nc.declare_dram_parameter("resid", [128, 8, D], F, isOutput=False)
    onescol = nc.declare_dram_parameter("onescol", [128, 1], BF, isOutput=False)
    ones1p = nc.declare_dram_parameter("ones1p", [1, 128], BF, isOutput=False)
    ones32 = nc.declare_dram_parameter("ones32", [128, 4], E4, isOutput=False)
    # salt: dummy input whose shape makes each build's HLO structurally unique,
    # defeating executable dedup layers that ignore backend_config
    salt_p = nc.declare_dram_parameter("salt", [1, salt], F, isOutput=False)
    out = nc.declare_dram_parameter("out", [H, D], F, isOutput=True)

    NJ = D // 256     # 4 contraction pair-tiles (2x128 rows each)
    NE = DQK // 128   # 2 e-tiles
    NK = S // 128     # 16 k-tiles
    NM = NK // 2      # 8 k pair-tiles
    QB = 512          # q-block
    NQB = H // QB     # 2 q-blocks per core
    NQT = QB // 128   # 4 q-tiles per block

    with tile.TileContext(nc) as tc:
        with (
            tc.tile_pool(name="consts", bufs=1) as cp,
            tc.tile_pool(name="qt_sb", bufs=NE) as qtp,
            tc.tile_pool(name="kt_sb", bufs=NE) as ktp,
            tc.tile_pool(name="vh_sb", bufs=NM) as vhp,
            tc.tile_pool(name="vl_sb", bufs=NM) as vlp,
        ):
            nbias = cp.tile([128, 1], F, tag="nbias")
            nc.gpsimd.memset(nbias[:], -EXP_SHIFT)
            # dummy exp right away so the ACT table load happens at t=0, not
            # in the middle of the first scores->exp stream
            scratch1 = cp.tile([128, 1], F, tag="scratch1")
            nc.scalar.activation(
                scratch1[:], nbias[:], mybir.ActivationFunctionType.Exp
            )
            ones_col = cp.tile([128, 1], BF, tag="ones_col")
            ones_row = cp.tile([1, 128], BF, tag="ones_row")
            ones32_sb = cp.tile([128, 2, 2], E4, tag="ones32")
            bqk_sb = cp.tile([128, 2, 2], F, tag="bqk")
            salt_sb = cp.tile([1, salt], F, tag="salt")

            def load_consts():
                # none of these are needed before ~15us in; keep them behind
                # the first critical x/wq loads in the DMA queue
                nc.sync.dma_start(ones_col[:], onescol[:])
                nc.sync.dma_start(ones_row[:], ones1p[:])
                nc.sync.dma_start(ones32_sb[:, :, :], ones32[:, :])
                for e in range(NE):
                    nc.sync.dma_start(
                        bqk_sb[:, e : e + 1, :], bqk[e * 128 : (e + 1) * 128, :]
                    )
                nc.sync.dma_start(salt_sb[:], salt_p[:])

            QT = [qtp.tile([128, H], R, tag="qt", name=f"QT{e}") for e in range(NE)]
            KT = [ktp.tile([128, S], R, tag="kt", name=f"KT{e}") for e in range(NE)]
            # V pair tiles: dim1 = k-tile parity for DoubleRow pairing
            Vh = [vhp.tile([128, 2, D], E4, tag="vh", name=f"Vh{m}") for m in range(NM)]
            Vl = [vlp.tile([128, 2, D], E4, tag="vl", name=f"Vl{m}") for m in range(NM)]

            def jsl(big, j, c0, c1):
                return big[:, 2 * j : 2 * j + 2, c0:c1]

            def load_inputs(xsp, wsp):
                """All inputs as a few big partition-major DMAs, ordered by
                first use: wq, first x chunk, wq_lo, ... (QKT runs first),
                then the rest of x, then Wv."""
                xhs = xsp.tile([128, 8, S], E4, tag="xh", name="xhs")
                xls = xsp.tile([128, 8, S], E4, tag="xl", name="xls")
                wvhs = wsp.tile([128, 8, D], E4, tag="wvh", name="wvhs")
                wvls = wsp.tile([128, 8, D], E4, tag="wvl", name="wvls")

                def chunk(dst, src, c0, c1):
                    nc.sync.dma_start(dst[:, :, c0:c1], src[:, :, c0:c1])

                wqhs = wsp.tile([128, 8, DQK], E4, tag="wqh", name="wqhs")
                nc.sync.dma_start(wqhs[:, :, :], wqh[:, :, :])
                chunk(xhs, xh, 0, 512)
                wqls = wsp.tile([128, 8, DQK], E4, tag="wql", name="wqls")
                nc.sync.dma_start(wqls[:, :, :], wql[:, :, :])
                chunk(xls, xl, 0, 512)
                wkhs = wsp.tile([128, 8, DQK], E4, tag="wkh", name="wkhs")
                nc.sync.dma_start(wkhs[:, :, :], wkh[:, :, :])
                wkls = wsp.tile([128, 8, DQK], E4, tag="wkl", name="wkls")
                nc.sync.dma_start(wkls[:, :, :], wkl[:, :, :])
                load_consts()
                for sb in range(1, 4):
                    chunk(xhs, xh, sb * 512, (sb + 1) * 512)
                    chunk(xls, xl, sb * 512, (sb + 1) * 512)
                for vb in range(2):
                    chunk(wvhs, wvh, vb * 512, (vb + 1) * 512)
                    chunk(wvls, wvl, vb * 512, (vb + 1) * 512)
                return xhs, xls, wqhs, wqls, wkhs, wkls, wvhs, wvls

            def proj_qkt(x2, w4, pqkp):
                """QT = (x[:H] @ Wq')^T + bq', KT = (x @ Wk')^T + bk'.
                blk-major so the x-chunk DMA stream stays ahead of the PE."""
                xhs, xls = x2
                wqhs, wqls, wkhs, wkls = w4
                for dst, nblk, wh_, wl_, bsel in (
                    (QT, NQB, wqhs, wqls, 0),
                    (KT, S // 512, wkhs, wkls, 1),
                ):
                    for blk in range(nblk):
                        for e in range(NE):
                            ps = pqkp.tile([128, 512], F, tag="pqk")
                            c = 0
                            for xs, ws in ((xhs, wh_), (xhs, wl_), (xls, wh_)):
                                for j in range(NJ):
                                    nc.tensor.matmul(
                                        ps[:],
                                        jsl(ws, j, e * 128, (e + 1) * 128),
                                        jsl(xs, j, blk * 512, (blk + 1) * 512),
                                        start=(c == 0),
                                        stop=(c == 3 * NJ - 1),
                                        perf_mode=DR,
                                    )
                                    c += 1
                            nc.vector.tensor_scalar_add(
                                dst[e][:, blk * 512 : (blk + 1) * 512],
                                ps[:],
                                bqk_sb[:, e : e + 1, bsel : bsel + 1].opt(),
                            )

            def compute_v(x2, wv2, pvp):
                """V = x @ Wv' (no bias: bv lives in resid), split e4m3 hi/lo.
                The V_lo subtract runs on the otherwise-idle Pool engine."""
                xhs, xls = x2
                wvhs, wvls = wv2
                for vb in range(2):
                    for kt in range(NK):
                        m, par = kt // 2, kt % 2
                        ps = pvp.tile([128, 512], F, tag="pv")
                        c = 0
                        for xs, ws in ((xhs, wvhs), (xhs, wvls), (xls, wvhs)):
                            for j in range(NJ):
                                nc.tensor.matmul(
                                    ps[:],
                                    jsl(xs, j, kt * 128, (kt + 1) * 128),
                                    jsl(ws, j, vb * 512, (vb + 1) * 512),
                                    start=(c == 0),
                                    stop=(c == 3 * NJ - 1),
                                    perf_mode=DR,
                                )
                                c += 1
                        vsl = (slice(None), slice(par, par + 1),
                               slice(vb * 512, (vb + 1) * 512))
                        nc.scalar.activation(
                            Vh[m][vsl], ps[:], mybir.ActivationFunctionType.Copy
                        )
                        nc.vector.scalar_tensor_tensor(
                            out=Vl[m][vsl],
                            in0=ps[:],
                            scalar=1.0,
                            in1=Vh[m][vsl],
                            op0=Alu.mult,
                            op1=Alu.subtract,
                        )

            def stage_scores(qb, pbfp, lrp, pst, prow):
                """scores -> exp -> l_row -> linv for one q-block."""
                pbf = []
                l_bank = prow.tile([128, 512], F, tag="rowbank", name=f"lbank{qb}")
                l_ps = l_bank[0:1, :]

                def l_row(kt):
                    nc.tensor.matmul(
                        l_ps,
                        ones_col[:],
                        pbf[kt][:],
                        start=(kt == 0),
                        stop=(kt == NK - 1),
                    )

                for kt in range(NK):
                    ps = pst.tile([128, QB], F, tag="st")
                    for e in range(NE):
                        nc.tensor.matmul(
                            ps[:],
                            KT[e][:, kt * 128 : (kt + 1) * 128],
                            QT[e][:, qb * QB : (qb + 1) * QB],
                            start=(e == 0),
                            stop=(e == NE - 1),
                        )
                    pt = pbfp.tile([128, QB], BF, tag="pbf", name=f"pbf{qb}_{kt}")
                    nc.scalar.activation(
                        pt[:],
                        ps[:],
                        mybir.ActivationFunctionType.Exp,
                        bias=nbias[:],
                        scale=1.0 / (WSC * WSC),
                    )
                    pbf.append(pt)
                    # skew the l_row accumulation one tile behind the scores
                    # so PE never waits on ACT mid-stream
                    if kt >= 1:
                        l_row(kt - 1)
                l_row(NK - 1)
                linv_row = lrp.tile([1, 512], F, tag="linv")
                nc.vector.reciprocal_approx_fast(linv_row[:], l_ps)
                linv_bf = lrp.tile([1, 512], BF, tag="linvbf")
                nc.vector.tensor_copy(out=linv_bf[:], in_=linv_row[:])
                return pbf, linv_bf

            def stage_phat(qb, pbf, linv_row, phpp, prow, lrp):
                """rank-1 broadcast of linv via the PE, then
                P_hat = e4m3(64 * P * linv) as DoubleRow pair tiles on the
                otherwise-idle Pool engine (via an SBUF copy of the
                broadcast: Pool cannot read PSUM)."""
                bc_ps = prow.tile([128, 512], F, tag="rowbank", name=f"bcbank{qb}")
                nc.tensor.matmul(bc_ps[:], ones_row[:], linv_row[:])
                php = [phpp.tile([128, 2, QB], E4, tag="php", name=f"php{qb}_{m}")
                       for m in range(NM)]
                for kt in range(NK):
                    nc.vector.scalar_tensor_tensor(
                        out=php[kt // 2][:, kt % 2 : kt % 2 + 1, :],
                        in0=pbf[kt][:],
                        scalar=64.0,
                        in1=bc_ps[:],
                        op0=Alu.mult,
                        op1=Alu.mult,
                    )
                return php

            def stage_b(qb, php, op, lip, plh, patt, resid_sb):
                """l_hat + attended + output for one q-block. The final
                normalize+residual runs on the Pool engine."""
                for qt in range(NQT):
                    qtg = qb * NQT + qt
                    qsl = (slice(None), slice(None), slice(qt * 128, (qt + 1) * 128))
                    # l_hat = 32 * sum_k P_hat  (the 32 folds away 1/WSC);
                    # interleaved with the hi-term attended matmuls so each
                    # loaded P_hat stationary serves 3 matmuls
                    lh_ps = plh.tile([128, 2], F, tag="lh")
                    att = [
                        patt.tile([128, 512], F, tag="att", name=f"att{vb}")
                        for vb in range(2)
                    ]
                    for m in range(NM):
                        nc.tensor.matmul(
                            lh_ps[:],
                            php[m][qsl],
                            ones32_sb[:],
                            start=(m == 0),
                            stop=(m == NM - 1),
                            perf_mode=DR,
                        )
                        for Vp in (Vh, Vl):
                            for vb in range(2):
                                nc.tensor.matmul(
                                    att[vb][:],
                                    php[m][qsl],
                                    Vp[m][:, :, vb * 512 : (vb + 1) * 512],
                                    start=(m == 0 and Vp is Vh),
                                    stop=(m == NM - 1 and Vp is Vl),
                                    perf_mode=DR,
                                )
                    l2inv = lip.tile([128, 1], F, tag="l2inv")
                    nc.vector.reciprocal(l2inv[:], lh_ps[:, 0:1])
                    o_t = op.tile([128, D], F, tag="o")
                    last = qtg == NQB * NQT - 1
                    for vb in range(2):
                        nc.vector.scalar_tensor_tensor(
                            out=o_t[:, vb * 512 : (vb + 1) * 512],
                            in0=att[vb][:],
                            scalar=l2inv[:],
                            in1=resid_sb[:, qtg : qtg + 1, vb * 512 : (vb + 1) * 512].opt(),
                            op0=Alu.mult,
                            op1=Alu.add,
                        )
                        if last:
                            # pipeline the very last store with its STT to
                            # shorten the drain tail
                            nc.sync.dma_start(
                                out[qtg * 128 : (qtg + 1) * 128,
                                    vb * 512 : (vb + 1) * 512],
                                o_t[:, vb * 512 : (vb + 1) * 512],
                            )
                    if not last:
                        nc.sync.dma_start(
                            out[qtg * 128 : (qtg + 1) * 128, :], o_t[:]
                        )

            for _rep in range(reps):
                if _rep > 0:
                    tc.strict_bb_all_engine_barrier()
                with (
                    tc.tile_pool(name="x_sb", bufs=1) as xsp,
                    tc.tile_pool(name="w_sb", bufs=1) as wsp,
                    tc.tile_pool(name="resid_sb", bufs=1) as rsp,
                    tc.tile_pool(name="pbf_sb", bufs=2 * NK) as pbfp,
                    tc.tile_pool(name="php_sb", bufs=2 * NM) as phpp,
                    tc.tile_pool(name="linv_sb", bufs=2) as lrp,
                    tc.tile_pool(name="o_sb", bufs=2) as op,
                    tc.tile_pool(name="l2_sb", bufs=2) as lip,
                ):
                    ins = load_inputs(xsp, wsp)
                    xhs, xls, wqhs, wqls, wkhs, wkls, wvhs, wvls = ins
                    resid_t = rsp.tile([128, 8, D], F, tag="resid", name="resids")
                    nc.sync.dma_start(resid_t[:, :, :], resid[:, :, :])
                    with tc.tile_pool(name="pqk", bufs=2, space="PSUM") as pqkp:
                        proj_qkt((xhs, xls), (wqhs, wqls, wkhs, wkls), pqkp)
                    with tc.tile_pool(name="prow", bufs=2, space="PSUM") as prow:
                        with tc.tile_pool(name="pst", bufs=3, space="PSUM") as pst:
                            pbf0, linv0 = stage_scores(0, pbfp, lrp, pst, prow)
                            pbf1, linv1 = stage_scores(1, pbfp, lrp, pst, prow)
                        # P_hat conversions (DVE) overlap the V matmuls below
                        php0 = stage_phat(0, pbf0, linv0, phpp, prow, lrp)
                        php1 = stage_phat(1, pbf1, linv1, phpp, prow, lrp)
                        with tc.tile_pool(name="pv", bufs=4, space="PSUM") as pvp:
                            compute_v((xhs, xls), (wvhs, wvls), pvp)
                    with (
                        tc.tile_pool(name="plh", bufs=1, space="PSUM") as plh,
                        tc.tile_pool(name="patt", bufs=4, space="PSUM") as patt,
                    ):
                        stage_b(0, php0, op, lip, plh, patt, resid_t)
                        stage_b(1, php1, op, lip, plh, patt, resid_t)

    nc.finalize()
    return nc


class _SpmdRunner:
    """Run a finalized Bass module on n_cores via PJRT (axon path)."""

    def __init__(self, nc, n_cores):
        import jax
        from jax.sharding import Mesh, PartitionSpec

        try:
            from jax.experimental.shard_map import shard_map
        except ImportError:
            from jax.shard_map import shard_map
        import concourse.mybir as mybir
        from concourse.bass2jax import (
            _bass_exec_p,
            install_neuronx_cc_hook,
            partition_id_tensor,
        )

        install_neuronx_cc_hook()
        self.jax = jax
        self.n_cores = n_cores
        partition_name = (
            nc.partition_id_tensor.name if nc.partition_id_tensor else None
        )
        in_names, out_names, out_avals, zero_outs = [], [], [], []
        for alloc in nc.m.functions[0].allocations:
            if not isinstance(alloc, mybir.MemoryLocationSet):
                continue
            name = alloc.memorylocations[0].name
            if alloc.kind == "ExternalInput":
                if name != partition_name:
                    in_names.append(name)
            elif alloc.kind == "ExternalOutput":
                out_names.append(name)
                shape = tuple(alloc.tensor_shape)
                dtype = mybir.dt.np(alloc.dtype)
                out_avals.append(jax.core.ShapedArray(shape, dtype))
                zero_outs.append(np.zeros(shape, dtype))
        self.in_names = in_names
        self.out_names = out_names
        self.out_avals = out_avals
        self.zero_outs = zero_outs
        n_params = len(in_names)
        n_outs = len(out_avals)
        all_in_names = list(in_names) + list(out_names)
        if partition_name is not None:
            all_in_names.append(partition_name)

        def _body(*args):
            operands = list(args)
            if partition_name is not None:
                operands.append(partition_id_tensor())
            outs = _bass_exec_p.bind(
                *operands,
                out_avals=tuple(out_avals),
                in_names=tuple(all_in_names),
                out_names=tuple(out_names),
                lowering_input_output_aliases=(),
                sim_require_finite=True,
                sim_require_nnan=True,
                nc=nc,
            )
            return tuple(outs)

        donate = tuple(range(n_params, n_params + n_outs))
        devices = jax.devices()[:n_cores]
        assert len(devices) == n_cores, (
            f"need {n_cores} devices, found {len(jax.devices())}"
        )
        mesh = Mesh(np.asarray(devices), ("core",))
        in_specs = (PartitionSpec("core"),) * (n_params + n_outs)
        out_specs = (PartitionSpec("core"),) * n_outs
        self.fn = jax.jit(
            shard_map(
                _body,
                mesh=mesh,
                in_specs=in_specs,
                out_specs=out_specs,
                check_rep=False,
            ),
            donate_argnums=donate,
            keep_unused=True,
        )

    def set_inputs(self, in_maps):
        n = len(self.in_names)
        per_core = [
            [np.ascontiguousarray(m[name]) for name in self.in_names]
            for m in in_maps
        ]
        concat_in = [
            np.concatenate([per_core[c][i] for c in range(self.n_cores)], axis=0)
            for i in range(n)
        ]
        self.dev_in = [self.jax.device_put(a) for a in concat_in]
        self.jax.block_until_ready(self.dev_in)

    def run(self, reuse_out=None):
        if reuse_out is None:
            outs = [
                np.zeros((self.n_cores * z.shape[0], *z.shape[1:]), z.dtype)
                for z in self.zero_outs
            ]
        else:
            outs = reuse_out
        outs = self.fn(*self.dev_in, *outs)
        self.jax.block_until_ready(outs)
        self._last = outs
        return outs

    def results(self):
        return [
            {
                name: np.asarray(self._last[i]).reshape(
                    self.n_cores, *self.out_avals[i].shape
                )[c]
                for i, name in enumerate(self.out_names)
            }
            for c in range(self.n_cores)
        ]


def _get_runner():
    global _RUNNER
    if _RUNNER is None:
        last = None
        for _attempt in range(3):
            try:
                nc = _build_kernel()
                break
            except Exception as e:  # rare Tile-scheduler deadlock flake
                last = e
        else:
            raise last
        _RUNNER = _SpmdRunner(nc, N_CORES)
    return _RUNNER


def _e4(a):
    import ml_dtypes

    return np.asarray(a, dtype=ml_dtypes.float8_e4m3)


def _split8(a):
    hi = _e4(a)
    lo = _e4(np.asarray(a, np.float32) - hi.astype(np.float32))
    return hi, lo


def _pmaj(a):
    """[128*k, n] -> [128, k, n] partition-major consolidation."""
    k = a.shape[0] // 128
    return np.ascontiguousarray(a.reshape(k, 128, a.shape[1]).transpose(1, 0, 2))


def host_inputs(x, Wq, bq, Wk, bk, Wv, bv, salt=3):
    """Build the per-core input maps (also used by test.py)."""
    import ml_dtypes

    x = np.ascontiguousarray(np.asarray(x, dtype=np.float32))
    Wq = np.asarray(Wq, np.float32) * WSC
    Wk = np.asarray(Wk, np.float32) * WSC
    Wv = np.asarray(Wv, np.float32) * WSC
    bqk = np.stack(
        [np.asarray(bq, np.float32) * WSC, np.asarray(bk, np.float32) * WSC], axis=1
    )  # [DQK, 2]
    bv = np.asarray(bv, np.float32)

    wq_hi, wq_lo = (_pmaj(w) for w in _split8(Wq))
    wk_hi, wk_lo = (_pmaj(w) for w in _split8(Wk))
    wv_hi, wv_lo = (_pmaj(w) for w in _split8(Wv))
    x_hi, x_lo = _split8(x)   # [B, S, D]

    onescol = np.ones((128, 1), ml_dtypes.bfloat16)
    ones1p = np.ones((1, 128), ml_dtypes.bfloat16)
    ones32 = np.full((128, 4), 32.0, ml_dtypes.float8_e4m3)
    saltz = np.zeros((1, salt), np.float32)

    in_maps = []
    for c in range(N_CORES):
        b, h = c // 2, c % 2
        rot = np.concatenate(
            [np.arange(h * H, (h + 1) * H), np.arange((1 - h) * H, (2 - h) * H)]
        )
        in_maps.append(
            {
                "xh": _pmaj(np.ascontiguousarray(x_hi[b][rot].T)),
                "xl": _pmaj(np.ascontiguousarray(x_lo[b][rot].T)),
                "wqh": wq_hi, "wql": wq_lo,
                "wkh": wk_hi, "wkl": wk_lo,
                "wvh": wv_hi, "wvl": wv_lo,
                "bqk": bqk,
                "resid": _pmaj(x[b, h * H : (h + 1) * H] + bv),
                "onescol": onescol, "ones1p": ones1p, "ones32": ones32,
                "salt": saltz,
            }
        )
    return in_maps


def kernel(x, Wq, bq, Wk, bk, Wv, bv):
    in_maps = host_inputs(x, Wq, bq, Wk, bk, Wv, bv)
    runner = _get_runner()
    runner.set_inputs(in_maps)
    runner.run()
    res = runner.results()
    outp = np.empty((B, S, D), np.float32)
    for c in range(N_CORES):
        b, h = c // 2, c % 2
        outp[b, h * H : (h + 1) * H] = res[c]["out"]
    return outp
